# revision 14
# baseline (speedup 1.0000x reference)
"""Bass/Trainium2 kernel for nn_EnhancedMultiHeadAttention (sparse_attention).

out[b,h,i,j] = softmax_j( (q_bh i . k_bh j) * sc + relbias[b,i,j] + mask_term[b,i,j] )
  q = query @ Wq.T + bq   (sc = 1/sqrt(64) folded into Wq/bq on host)
  relbias[b,i,j] = (mean_h q[b,h,i,:]) . rel_k_table[clip(j-i,-128,128)+128, :] * sc
  mask_term = 0 where mask==1 else -3e4

Sharding: 8 cores = 4 batches x 2 head-halves (8 heads per core).

Measured-engine-cost-driven design (HW-profiled rates):
  - exp is ACT-only at ~1 elem/cycle/lane @1.2GHz -> the 8 Mi elem/core exp
    pass (~67us) + per-tile accumulator reads set the pace; everything else
    is organized to hide under it.
  - The row sum comes free from ACT's accum_out (Vector-side reductions
    measured 1134-1219ns/tile -- too slow), which forces the mask+rel bias
    to be ADDITIVE in PSUM before exp.
  - Bias injection and QK scores use K=64 matmuls issued in alternating
    PE row groups (tile_position (0,0)/(64,0), distinct PSUM banks per
    pair) -- measured ~2x concurrency. The bias identity is split into
    two K=64 halves (slices of the 128x128 identity) to enable this.
  - fp16 everywhere off-PSUM (incl. the DRAM output -> halves the
    dominant DMA stream); Vector only does reciprocal + one 335ns
    normalize-multiply per tile.
  - Input DMAs are ordered/chunked so the first head-pair's projections,
    the rel-bias band (qm -> W -> DRAM-skew bounce) and comb[0] are all
    ready ~22us in; projections for pair t+1 are emitted between row
    tiles of pair t to fill ACT-bound PE gaps.
"""

import numpy as np

B, S, D, H = 4, 1024, 1024, 16
DK = 64          # head dim
MAXREL = 128
NREL = 2 * MAXREL + 1          # 257
WPADW = 2 * MAXREL + NREL - 2  # 511 = 127 + 257 + 127
NRELP = 260     # rel matmul free dim padded for ISA restrictions
HPC = 8          # heads per core
DHALF = 512      # projected dims per core
NCORES = 8
PT = 128         # partition tile
NT = S // PT     # 8 row tiles

MASKV = 30000.0  # fp16-safe large negative bias for masked entries

_CACHE = {}


def _build():
    from contextlib import ExitStack

    import concourse.bass as bass
    import concourse.mybir as mybir
    import concourse.tile as tile
    from concourse import bacc
    from concourse.tile import add_dep_helper

    F32 = mybir.dt.float32
    F16 = mybir.dt.float16
    I8 = mybir.dt.int8
    AF = mybir.ActivationFunctionType

    nc = bacc.Bacc("TRN2", target_bir_lowering=False, debug=False)

    xT = nc.dram_tensor("xT", [D, S], F16, kind="ExternalInput")
    kTx = nc.dram_tensor("kTx", [D, S], F16, kind="ExternalInput")
    maskf = nc.dram_tensor("maskf", [S, S], F16, kind="ExternalInput")
    wqT = nc.dram_tensor("wqT", [D, DHALF], F16, kind="ExternalInput")
    wkT = nc.dram_tensor("wkT", [D, DHALF], F16, kind="ExternalInput")
    bq4 = nc.dram_tensor("bq4", [PT, 4], F32, kind="ExternalInput")
    bk4 = nc.dram_tensor("bk4", [PT, 4], F32, kind="ExternalInput")
    wmT = nc.dram_tensor("wmT", [D, DK], F16, kind="ExternalInput")
    bm1 = nc.dram_tensor("bm1", [DK, 1], F32, kind="ExternalInput")
    tT = nc.dram_tensor("tT", [DK, DHALF], F16, kind="ExternalInput")
    out_d = nc.dram_tensor("out", [HPC, S, S], F16, kind="ExternalOutput")
    wpad_d = nc.dram_tensor("wpad_scratch", [S, WPADW], F16)
    ident_d = nc.inline_tensor(np.eye(PT, dtype=np.float16), "ident")

    with tile.TileContext(nc) as tc, ExitStack() as ctx:
        persist = ctx.enter_context(tc.tile_pool(name="persist", bufs=1))
        bpool = ctx.enter_context(tc.tile_pool(name="bpool", bufs=1))
        wppool = ctx.enter_context(tc.tile_pool(name="wppool", bufs=8))
        epool = ctx.enter_context(tc.tile_pool(name="epool", bufs=4))
        opool = ctx.enter_context(tc.tile_pool(name="opool", bufs=4))
        spool = ctx.enter_context(tc.tile_pool(name="spool", bufs=8))
        psum = ctx.enter_context(tc.tile_pool(name="psum", bufs=3, space="PSUM"))
        psump = ctx.enter_context(tc.tile_pool(name="psump", bufs=2, space="PSUM"))

        # ---- small constants ----
        id_sb = persist.tile([PT, PT], F16, tag="ident")
        nc.sync.dma_start(id_sb[:], ident_d[:])
        bq_sb = persist.tile([PT, 4], F32, tag="bq")
        nc.sync.dma_start(bq_sb[:], bq4[:])
        bk_sb = persist.tile([PT, 4], F32, tag="bk")
        nc.sync.dma_start(bk_sb[:], bk4[:])
        bm_sb = persist.tile([DK, 1], F32, tag="bm")
        nc.sync.dma_start(bm_sb[:], bm1[:])
        tT_sb = persist.tile([DK, DHALF], F16, tag="tT")
        nc.sync.dma_start(tT_sb[:], tT[:])

        # ---- PE warmup while the first input DMAs land; tiny exp issued
        # first so the ~2.7us ACT table load happens off the critical path ----
        warm_sb = persist.tile([PT, DHALF], F16, tag="warm")
        nc.vector.memset(warm_sb[:], 0.0)
        dume = persist.tile([PT, 1], F16, tag="dume")
        nc.scalar.activation(dume[:], warm_sb[:, 0:1], AF.Exp, bias=0.0,
                             scale=1.0)
        wps = psump.tile([PT, DHALF], F32, tag="psp", name="warmps")
        for i in range(16):
            nc.tensor.matmul(wps[:], id_sb[:], warm_sb[:], start=True, stop=True)

        # ---- bulk input loads; issue order sets DMA priority, kc/row
        # chunking lets compute chase the transfers ----
        def load_all(pool_, name, dram, width, dt_, parts=1):
            t = pool_.tile([PT, NT * width], dt_, tag=name, name=name)
            cpp = NT // parts
            for pi in range(parts):
                srcap = bass.AP(dram, pi * cpp * PT * width,
                                [[width, PT], [PT * width, cpp], [1, width]])
                nc.sync.dma_start(
                    t[:, pi * cpp * width:(pi + 1) * cpp * width]
                    .rearrange("p (c s) -> p c s", s=width), srcap)
            return t

        wm_all = load_all(persist, "wm_all", wmT, DK, F16)
        x_all = load_all(persist, "x_all", xT, S, F16, parts=8)
        wq_all = persist.tile([PT, NT * DHALF], F16, tag="wq_all", name="wq_all")
        wk_all = persist.tile([PT, NT * DHALF], F16, tag="wk_all", name="wk_all")

        def load_w_cols(t_sb, dram, tpair):
            # load columns [tpair*128, (tpair+1)*128) of a [D, DHALF] weight:
            # chunk kc of the sbuf tile holds dram rows kc*128.. as [128, 512]
            srcap = bass.AP(dram, tpair * PT,
                            [[DHALF, PT], [PT * DHALF, NT], [1, PT]])
            dst = t_sb[:].rearrange("p (c h q) -> p c h q", h=4, q=PT)[:, :, tpair]
            nc.sync.dma_start(dst, srcap)

        load_w_cols(wq_all, wqT, 0)
        load_w_cols(wk_all, wkT, 0)
        # comb[m] starts life as the host-prescaled fp16 mask term
        # (mask-1)*MASKV; band + clipped-edge rel bias are added in place
        comb_sb = [persist.tile([PT, S], F16, tag=f"comb{m}", name=f"comb{m}")
                   for m in range(NT)]

        def load_maskf(m):
            nc.sync.dma_start(comb_sb[m][:],
                              bass.AP(maskf, m * PT * S, [[S, PT], [1, S]]))

        load_maskf(0)
        load_maskf(1)
        k_all = load_all(persist, "k_all", kTx, S, F16, parts=8)
        load_w_cols(wq_all, wqT, 1)
        load_w_cols(wk_all, wkT, 1)
        for m in range(2, NT):
            load_maskf(m)
        for tp in range(2, 4):
            load_w_cols(wq_all, wqT, tp)
        for tp in range(2, 4):
            load_w_cols(wk_all, wkT, tp)

        x_tiles = [x_all[:, kc * S:(kc + 1) * S] for kc in range(NT)]
        k_tiles = [k_all[:, kc * S:(kc + 1) * S] for kc in range(NT)]
        wm_sb = [wm_all[:, kc * DK:(kc + 1) * DK] for kc in range(NT)]
        wq_tiles = [wq_all[:, kc * DHALF:(kc + 1) * DHALF] for kc in range(NT)]
        wk_tiles = [wk_all[:, kc * DHALF:(kc + 1) * DHALF] for kc in range(NT)]

        # ---- head-mean projection qmT[64, S] ----
        qmT_sb = persist.tile([DK, S], F16, tag="qmT")
        for nh in range(2):
            nhs = slice(nh * DHALF, (nh + 1) * DHALF)
            ps = psump.tile([DK, DHALF], F32, tag="psp", name=f"qmps{nh}")
            for kc in range(NT):
                nc.tensor.matmul(ps[:], wm_sb[kc][:], x_tiles[kc][:, nhs],
                                 start=(kc == 0), stop=(kc == NT - 1))
            nc.vector.tensor_scalar_add(qmT_sb[:, nhs], ps[:], bm_sb[:])

        # ---- rel-bias band via W = qm @ T.T, edge-padded, DRAM skew ----
        band_info = [(max(0, PT * (m - 1)), min(S, PT * (m + 2)))
                     for m in range(NT)]
        w0_sb = [persist.tile([PT, 1], F32, tag=f"w0_{m}", name=f"w0_{m}")
                 for m in range(NT)]
        w256_sb = [persist.tile([PT, 1], F32, tag=f"w256_{m}",
                                name=f"w256_{m}") for m in range(NT)]
        # Phase A: all W matmuls + wpad fills + DRAM writes (no roundtrip
        # stalls inside the Vector/GpSimd FIFOs)
        # the host ships rel_k_table pre-padded with replicated edge
        # columns, so W = qm @ T_ext.T directly yields the clip-padded row
        wr_insts = []
        for m in range(NT):
            ps = psum.tile([PT, S], F32, tag="ps", name=f"wps{m}")
            ps = ps[:, 0:DHALF]
            nc.tensor.matmul(ps[:], qmT_sb[:, m * PT:(m + 1) * PT], tT_sb[:],
                             start=True, stop=True)
            wp = wppool.tile([PT, WPADW], F16, tag="wpad", name=f"wpad{m}")
            nc.vector.tensor_copy(wp[:], ps[:, 0:WPADW])
            nc.vector.tensor_copy(w0_sb[m][:], ps[:, MAXREL - 1:MAXREL])
            nc.vector.tensor_copy(w256_sb[m][:],
                                  ps[:, MAXREL + NREL - 2:MAXREL + NREL - 1])
            wr_insts.append(nc.gpsimd.dma_start(wpad_d[m * PT:(m + 1) * PT, :],
                                                wp[:]))
        # Phase B: all band skew-reads, pipelined behind their writes
        # band[p, jj] = wpad[m*128+p, (jlo+jj)-(m*128+p)+255]
        bt_sb = []
        for m in range(NT):
            jlo, jhi = band_info[m]
            bt = bpool.tile([PT, jhi - jlo], F16, tag=f"band{m}",
                            name=f"band{m}")
            srcap = bass.AP(wpad_d, PT * (WPADW - 1) * m + jlo + (WPADW // 2),
                            [[WPADW - 1, PT], [1, jhi - jlo]])
            ri = nc.gpsimd.dma_start(bt[:], srcap)
            add_dep_helper(ri.ins, wr_insts[m].ins, reason="wpad DRAM RAW")
            bt_sb.append(bt)

        # ---- q/k projections (half-width chunks -> 1-bank prep psums) ----
        qT_sb = [persist.tile([PT, S], F16, tag=f"qT{i}", name=f"qT{i}")
                 for i in range(4)]
        # kT stored as two zero-padded copies so the QK matmuls can use the
        # full-K (128x128) PE mode: avoids 64<->128 tiling-mode switches
        # (PE drains) between QK and the identity bias matmuls
        kTA_sb = [persist.tile([PT, S], F16, tag=f"kTA{i}", name=f"kTA{i}")
                  for i in range(4)]
        kTB_sb = [persist.tile([PT, S], F16, tag=f"kTB{i}", name=f"kTB{i}")
                  for i in range(4)]
        for i in range(4):
            nc.vector.memset(kTA_sb[i][DK:PT, :], 0.0)
            nc.vector.memset(kTB_sb[i][0:DK, :], 0.0)

        def project_half(t, nh, w_tiles, x_t, dst, bias_sb, pstag):
            nhs = slice(nh * DHALF, (nh + 1) * DHALF)
            ps = psump.tile([PT, DHALF], F32, tag="psp",
                            name=f"proj{pstag}{t}_{nh}")
            for kc in range(NT):
                nc.tensor.matmul(ps[:], w_tiles[kc][:, t * PT:(t + 1) * PT],
                                 x_t[kc][:, nhs],
                                 start=(kc == 0), stop=(kc == NT - 1))
            if isinstance(dst, tuple):  # (kTA, kTB) split evacuation
                nc.vector.tensor_scalar_add(dst[0][0:DK, nhs], ps[0:DK, :],
                                            bias_sb[0:DK, t:t + 1])
                nc.vector.tensor_scalar_add(dst[1][DK:PT, nhs], ps[DK:PT, :],
                                            bias_sb[DK:PT, t:t + 1])
            else:
                nc.vector.tensor_scalar_add(dst[:, nhs], ps[:],
                                            bias_sb[:, t:t + 1])

        for nh in range(2):
            project_half(0, nh, wq_tiles, x_tiles, qT_sb[0], bq_sb, "q")
        for nh in range(2):
            project_half(0, nh, wk_tiles, k_tiles,
                         (kTA_sb[0], kTB_sb[0]), bk_sb, "k")

        # Phase C: comb[m] = (mask-1)*MASKV + relbias (band + clipped edges);
        # emitted after proj0 so its band-read waits don't head-of-line-block
        # the projection evacuations in the Vector FIFO
        with tc.tile_wait_until(0.025):
            for m in range(NT):
                jlo, jhi = band_info[m]
                cb = comb_sb[m]
                nc.vector.tensor_add(cb[:, jlo:jhi], cb[:, jlo:jhi],
                                     bt_sb[m][:])
                if jlo > 0:
                    nc.vector.tensor_scalar_add(cb[:, 0:jlo], cb[:, 0:jlo],
                                                w0_sb[m][:])
                if jhi < S:
                    nc.vector.tensor_scalar_add(cb[:, jhi:S], cb[:, jhi:S],
                                                w256_sb[m][:])

        # ---- main loop: K=64 QK matmuls in alternating PE row groups
        # (ABAB order, distinct PSUM banks -> concurrent pairs); comb bias
        # accumulated via full K=128 identity matmuls (mixing row groups
        # inside one accumulation group crashes TRN2, so the injection
        # stays in 128x128 mode like the baseline) ----
        for t in range(4):
            for m in range(NT):
                mb = slice(m * PT, (m + 1) * PT)
                psA = psum.tile([PT, S], F32, tag="ps", name=f"psA{t}_{m}")
                psB = psum.tile([PT, S], F32, tag="ps", name=f"psB{t}_{m}")
                n0, n1 = slice(0, DHALF), slice(DHALF, S)
                cb = comb_sb[m]
                nc.tensor.matmul(psA[:, n0], qT_sb[t][:, mb], kTA_sb[t][:, n0],
                                 start=True, stop=False)
                nc.tensor.matmul(psB[:, n0], qT_sb[t][:, mb], kTB_sb[t][:, n0],
                                 start=True, stop=False)
                nc.tensor.matmul(psA[:, n1], qT_sb[t][:, mb], kTA_sb[t][:, n1],
                                 start=True, stop=False)
                nc.tensor.matmul(psB[:, n1], qT_sb[t][:, mb], kTB_sb[t][:, n1],
                                 start=True, stop=False)
                nc.tensor.matmul(psA[:, n0], id_sb[:], cb[:, n0],
                                 start=False, stop=True)
                nc.tensor.matmul(psB[:, n0], id_sb[:], cb[:, n0],
                                 start=False, stop=True)
                nc.tensor.matmul(psA[:, n1], id_sb[:], cb[:, n1],
                                 start=False, stop=True)
                nc.tensor.matmul(psB[:, n1], id_sb[:], cb[:, n1],
                                 start=False, stop=True)
                for hi_i, psx in ((0, psA), (1, psB)):
                    h = 2 * t + hi_i
                    e = epool.tile([PT, S], F16, tag="e", name=f"e{h}_{m}")
                    sm = spool.tile([PT, 1], F32, tag="s", name=f"s{h}_{m}")
                    nc.scalar.activation(e[:], psx[:], AF.Exp, bias=0.0,
                                         scale=1.0, accum_out=sm[:])
                    r = spool.tile([PT, 1], F32, tag="r", name=f"r{h}_{m}")
                    nc.vector.reciprocal(r[:], sm[:])
                    o = opool.tile([PT, S], F16, tag="o", name=f"o{h}_{m}")
                    nc.vector.tensor_scalar_mul(o[:], e[:], r[:])
                    nc.sync.dma_start(out_d[h, mb, :], o[:])
                # next pair's projections fill PE gaps (4 chunks per t-loop)
                if t < 3 and m in (1, 3, 5, 7):
                    ci = (m - 1) // 2
                    if ci < 2:
                        project_half(t + 1, ci, wq_tiles, x_tiles,
                                     qT_sb[t + 1], bq_sb, "q")
                    else:
                        project_half(t + 1, ci - 2, wk_tiles, k_tiles,
                                     (kTA_sb[t + 1], kTB_sb[t + 1]), bk_sb,
                                     "k")

    nc.compile()
    return nc


def _get_nc():
    if "nc" not in _CACHE:
        _CACHE["nc"] = _build()
    return _CACHE["nc"]


def _prep_inputs(query, key, mask, Wq, bq, Wk, bk, rel_k_table):
    """Host-side sharding prep -> 8 per-core input dicts."""
    sc = 1.0 / np.sqrt(np.float32(DK))
    query = np.asarray(query, dtype=np.float32)
    key = np.asarray(key, dtype=np.float32)
    maskf = np.ascontiguousarray(
        ((np.asarray(mask) != 0).astype(np.float32) - 1.0) * MASKV
    ).astype(np.float16)
    Wq = np.asarray(Wq, dtype=np.float32)
    bq = np.asarray(bq, dtype=np.float32)
    Wk = np.asarray(Wk, dtype=np.float32)
    bk = np.asarray(bk, dtype=np.float32)
    T = np.asarray(rel_k_table, dtype=np.float32)

    WqTs = np.ascontiguousarray((Wq * sc).T)       # [D, D]
    WkT = np.ascontiguousarray(Wk.T)               # [D, D]
    bqs = bq * sc
    Wm16 = np.ascontiguousarray(((Wq.reshape(H, DK, D).mean(0) * sc).T).astype(np.float16))
    bm = (bq.reshape(H, DK).mean(0) * sc).reshape(DK, 1).astype(np.float32)
    tTc16 = np.zeros((DK, DHALF), np.float16)  # [64, 512] edge-padded
    tTc16[:, 0:MAXREL - 1] = T.T[:, 0:1].astype(np.float16)
    tTc16[:, MAXREL - 1:MAXREL - 1 + NREL] = T.T.astype(np.float16)
    tTc16[:, MAXREL - 1 + NREL:WPADW] = T.T[:, NREL - 1:NREL].astype(np.float16)

    xT = [np.ascontiguousarray(query[b].T.astype(np.float16)) for b in range(B)]
    kT = [np.ascontiguousarray(key[b].T.astype(np.float16)) for b in range(B)]

    in_maps = []
    for c in range(NCORES):
        b, hh = divmod(c, 2)
        cols = slice(hh * DHALF, (hh + 1) * DHALF)
        in_maps.append(dict(
            xT=xT[b], kTx=kT[b], maskf=maskf[b],
            wqT=np.ascontiguousarray(WqTs[:, cols].astype(np.float16)),
            wkT=np.ascontiguousarray(WkT[:, cols].astype(np.float16)),
            bq4=np.ascontiguousarray(bqs[cols].reshape(4, PT).T),
            bk4=np.ascontiguousarray(bk[cols].reshape(4, PT).T),
            wmT=Wm16, bm1=bm, tT=tTc16,
        ))
    return in_maps


def run(inputs: dict, trace: bool = False):
    from concourse.bass_utils import run_bass_kernel_spmd

    nc = _get_nc()
    in_maps = _prep_inputs(**inputs)
    res = run_bass_kernel_spmd(nc, in_maps, core_ids=list(range(NCORES)),
                               trace=trace)
    out = np.empty((B, H, S, S), dtype=np.float32)
    for c in range(NCORES):
        b, hh = divmod(c, 2)
        out[b, hh * HPC:(hh + 1) * HPC] = res.results[c]["out"]
    return out, res


def kernel(**inputs) -> np.ndarray:
    out, _ = run(inputs)
    return out


# revision 17
# speedup vs baseline: 1.0291x; 1.0291x over previous
"""Bass/Trainium2 kernel for nn_EnhancedMultiHeadAttention (sparse_attention).

out[b,h,i,j] = softmax_j( (q_bh i . k_bh j) * sc + relbias[b,i,j] + mask_term[b,i,j] )
  q = query @ Wq.T + bq   (sc = 1/sqrt(64) folded into Wq/bq on host)
  relbias[b,i,j] = (mean_h q[b,h,i,:]) . rel_k_table[clip(j-i,-128,128)+128, :] * sc
  mask_term = 0 where mask==1 else -3e4

Sharding: 8 cores = 4 batches x 2 head-halves (8 heads per core).

Measured-engine-cost-driven design (HW-profiled rates):
  - exp is ACT-only at ~1 elem/cycle/lane @1.2GHz -> the 8 Mi elem/core exp
    pass (~67us) + per-tile accumulator reads set the pace; everything else
    is organized to hide under it.
  - The row sum comes free from ACT's accum_out (Vector-side reductions
    measured 1134-1219ns/tile -- too slow), which forces the mask+rel bias
    to be ADDITIVE in PSUM before exp.
  - Bias injection and QK scores use K=64 matmuls issued in alternating
    PE row groups (tile_position (0,0)/(64,0), distinct PSUM banks per
    pair) -- measured ~2x concurrency. The bias identity is split into
    two K=64 halves (slices of the 128x128 identity) to enable this.
  - fp16 everywhere off-PSUM (incl. the DRAM output -> halves the
    dominant DMA stream); Vector only does reciprocal + one 335ns
    normalize-multiply per tile.
  - Input DMAs are ordered/chunked so the first head-pair's projections,
    the rel-bias band (qm -> W -> DRAM-skew bounce) and comb[0] are all
    ready ~22us in; projections for pair t+1 are emitted between row
    tiles of pair t to fill ACT-bound PE gaps.
"""

import numpy as np

B, S, D, H = 4, 1024, 1024, 16
DK = 64          # head dim
MAXREL = 128
NREL = 2 * MAXREL + 1          # 257
WPADW = 2 * MAXREL + NREL - 2  # 511 = 127 + 257 + 127
NRELP = 260     # rel matmul free dim padded for ISA restrictions
HPC = 8          # heads per core
DHALF = 512      # projected dims per core
NCORES = 8
PT = 128         # partition tile
NT = S // PT     # 8 row tiles

MASKV = 30000.0  # fp16-safe large negative bias for masked entries

_CACHE = {}


def _build():
    from contextlib import ExitStack

    import concourse.bass as bass
    import concourse.mybir as mybir
    import concourse.tile as tile
    from concourse import bacc
    from concourse.tile import add_dep_helper

    F32 = mybir.dt.float32
    F16 = mybir.dt.float16
    I8 = mybir.dt.int8
    AF = mybir.ActivationFunctionType

    nc = bacc.Bacc("TRN2", target_bir_lowering=False, debug=False)

    xT = nc.dram_tensor("xT", [D, S], F16, kind="ExternalInput")
    kTx = nc.dram_tensor("kTx", [D, S], F16, kind="ExternalInput")
    maskf = nc.dram_tensor("maskf", [S, S], F16, kind="ExternalInput")
    wqT = nc.dram_tensor("wqT", [D, DHALF], F16, kind="ExternalInput")
    wkT = nc.dram_tensor("wkT", [D, DHALF], F16, kind="ExternalInput")
    bq4 = nc.dram_tensor("bq4", [PT, 4], F32, kind="ExternalInput")
    bk4 = nc.dram_tensor("bk4", [PT, 4], F32, kind="ExternalInput")
    wmT = nc.dram_tensor("wmT", [D, DK], F16, kind="ExternalInput")
    bm1 = nc.dram_tensor("bm1", [DK, 1], F32, kind="ExternalInput")
    tT = nc.dram_tensor("tT", [DK, DHALF], F16, kind="ExternalInput")
    out_d = nc.dram_tensor("out", [HPC, S, S], F16, kind="ExternalOutput")
    wpad_d = nc.dram_tensor("wpad_scratch", [S, WPADW], F16)
    ident_d = nc.inline_tensor(np.eye(PT, dtype=np.float16), "ident")

    with tile.TileContext(nc) as tc, ExitStack() as ctx:
        persist = ctx.enter_context(tc.tile_pool(name="persist", bufs=1))
        bpool = ctx.enter_context(tc.tile_pool(name="bpool", bufs=1))
        wppool = ctx.enter_context(tc.tile_pool(name="wppool", bufs=8))
        epool = ctx.enter_context(tc.tile_pool(name="epool", bufs=4))
        opool = ctx.enter_context(tc.tile_pool(name="opool", bufs=4))
        spool = ctx.enter_context(tc.tile_pool(name="spool", bufs=8))
        psum = ctx.enter_context(tc.tile_pool(name="psum", bufs=3, space="PSUM"))
        psump = ctx.enter_context(tc.tile_pool(name="psump", bufs=2, space="PSUM"))

        # ---- small constants ----
        id_sb = persist.tile([PT, PT], F16, tag="ident")
        nc.sync.dma_start(id_sb[:], ident_d[:])
        bq_sb = persist.tile([PT, 4], F32, tag="bq")
        nc.sync.dma_start(bq_sb[:], bq4[:])
        bk_sb = persist.tile([PT, 4], F32, tag="bk")
        nc.sync.dma_start(bk_sb[:], bk4[:])
        bm_sb = persist.tile([DK, 1], F32, tag="bm")
        nc.sync.dma_start(bm_sb[:], bm1[:])
        tT_sb = persist.tile([DK, DHALF], F16, tag="tT")
        nc.sync.dma_start(tT_sb[:], tT[:])

        # ---- PE warmup while the first input DMAs land; tiny exp issued
        # first so the ~2.7us ACT table load happens off the critical path ----
        warm_sb = persist.tile([PT, DHALF], F16, tag="warm")
        nc.vector.memset(warm_sb[:], 0.0)
        dume = persist.tile([PT, 1], F16, tag="dume")
        nc.scalar.activation(dume[:], warm_sb[:, 0:1], AF.Exp, bias=0.0,
                             scale=1.0)
        wps = psump.tile([PT, DHALF], F32, tag="psp", name="warmps")
        for i in range(24):
            nc.tensor.matmul(wps[:], id_sb[:], warm_sb[:], start=True, stop=True)

        # ---- bulk input loads; issue order sets DMA priority, kc/row
        # chunking lets compute chase the transfers ----
        def load_all(pool_, name, dram, width, dt_, parts=1):
            t = pool_.tile([PT, NT * width], dt_, tag=name, name=name)
            cpp = NT // parts
            for pi in range(parts):
                srcap = bass.AP(dram, pi * cpp * PT * width,
                                [[width, PT], [PT * width, cpp], [1, width]])
                nc.sync.dma_start(
                    t[:, pi * cpp * width:(pi + 1) * cpp * width]
                    .rearrange("p (c s) -> p c s", s=width), srcap)
            return t

        wm_all = load_all(persist, "wm_all", wmT, DK, F16)
        x_all = load_all(persist, "x_all", xT, S, F16, parts=8)
        wq_all = persist.tile([PT, NT * DHALF], F16, tag="wq_all", name="wq_all")
        wk_all = persist.tile([PT, NT * DHALF], F16, tag="wk_all", name="wk_all")

        def load_w_cols(t_sb, dram, tpair):
            # load columns [tpair*128, (tpair+1)*128) of a [D, DHALF] weight:
            # chunk kc of the sbuf tile holds dram rows kc*128.. as [128, 512]
            srcap = bass.AP(dram, tpair * PT,
                            [[DHALF, PT], [PT * DHALF, NT], [1, PT]])
            dst = t_sb[:].rearrange("p (c h q) -> p c h q", h=4, q=PT)[:, :, tpair]
            nc.sync.dma_start(dst, srcap)

        load_w_cols(wq_all, wqT, 0)
        load_w_cols(wk_all, wkT, 0)
        # comb[m] starts life as the host-prescaled fp16 mask term
        # (mask-1)*MASKV; band + clipped-edge rel bias are added in place
        comb_sb = [persist.tile([PT, S], F16, tag=f"comb{m}", name=f"comb{m}")
                   for m in range(NT)]

        def load_maskf(m):
            nc.sync.dma_start(comb_sb[m][:],
                              bass.AP(maskf, m * PT * S, [[S, PT], [1, S]]))

        k_all = load_all(persist, "k_all", kTx, S, F16, parts=8)
        load_maskf(0)
        load_maskf(1)
        load_w_cols(wq_all, wqT, 1)
        load_w_cols(wk_all, wkT, 1)
        for m in range(2, NT):
            load_maskf(m)
        for tp in range(2, 4):
            load_w_cols(wq_all, wqT, tp)
        for tp in range(2, 4):
            load_w_cols(wk_all, wkT, tp)

        x_tiles = [x_all[:, kc * S:(kc + 1) * S] for kc in range(NT)]
        k_tiles = [k_all[:, kc * S:(kc + 1) * S] for kc in range(NT)]
        wm_sb = [wm_all[:, kc * DK:(kc + 1) * DK] for kc in range(NT)]
        wq_tiles = [wq_all[:, kc * DHALF:(kc + 1) * DHALF] for kc in range(NT)]
        wk_tiles = [wk_all[:, kc * DHALF:(kc + 1) * DHALF] for kc in range(NT)]

        # ---- head-mean projection qmT[64, S] ----
        qmT_sb = persist.tile([DK, S], F16, tag="qmT")
        for nh in range(2):
            nhs = slice(nh * DHALF, (nh + 1) * DHALF)
            ps = psump.tile([DK, DHALF], F32, tag="psp", name=f"qmps{nh}")
            for kc in range(NT):
                nc.tensor.matmul(ps[:], wm_sb[kc][:], x_tiles[kc][:, nhs],
                                 start=(kc == 0), stop=(kc == NT - 1))
            nc.vector.tensor_scalar_add(qmT_sb[:, nhs], ps[:], bm_sb[:])

        # ---- q/k projections (half-width chunks -> 1-bank prep psums) ----
        qT_sb = [persist.tile([PT, S], F16, tag=f"qT{i}", name=f"qT{i}")
                 for i in range(4)]
        # kT stored as two zero-padded copies so the QK matmuls can use the
        # full-K (128x128) PE mode: avoids 64<->128 tiling-mode switches
        # (PE drains) between QK and the identity bias matmuls
        kTA_sb = [persist.tile([PT, S], F16, tag=f"kTA{i}", name=f"kTA{i}")
                  for i in range(4)]
        kTB_sb = [persist.tile([PT, S], F16, tag=f"kTB{i}", name=f"kTB{i}")
                  for i in range(4)]
        for i in range(4):
            nc.vector.memset(kTA_sb[i][DK:PT, :], 0.0)
            nc.vector.memset(kTB_sb[i][0:DK, :], 0.0)

        def project_half(t, nh, w_tiles, x_t, dst, bias_sb, pstag):
            nhs = slice(nh * DHALF, (nh + 1) * DHALF)
            ps = psump.tile([PT, DHALF], F32, tag="psp",
                            name=f"proj{pstag}{t}_{nh}")
            for kc in range(NT):
                nc.tensor.matmul(ps[:], w_tiles[kc][:, t * PT:(t + 1) * PT],
                                 x_t[kc][:, nhs],
                                 start=(kc == 0), stop=(kc == NT - 1))
            if isinstance(dst, tuple):  # (kTA, kTB) split evacuation
                nc.vector.tensor_scalar_add(dst[0][0:DK, nhs], ps[0:DK, :],
                                            bias_sb[0:DK, t:t + 1])
                nc.vector.tensor_scalar_add(dst[1][DK:PT, nhs], ps[DK:PT, :],
                                            bias_sb[DK:PT, t:t + 1])
            else:
                nc.vector.tensor_scalar_add(dst[:, nhs], ps[:],
                                            bias_sb[:, t:t + 1])

        for nh in range(2):
            project_half(0, nh, wq_tiles, x_tiles, qT_sb[0], bq_sb, "q")
        for nh in range(2):
            project_half(0, nh, wk_tiles, k_tiles,
                         (kTA_sb[0], kTB_sb[0]), bk_sb, "k")

        # ---- rel-bias band via W = qm @ T.T, edge-padded, DRAM skew ----
        band_info = [(max(0, PT * (m - 1)), min(S, PT * (m + 2)))
                     for m in range(NT)]
        w0_sb = [persist.tile([PT, 1], F32, tag=f"w0_{m}", name=f"w0_{m}")
                 for m in range(NT)]
        w256_sb = [persist.tile([PT, 1], F32, tag=f"w256_{m}",
                                name=f"w256_{m}") for m in range(NT)]
        # Phase A: all W matmuls + wpad fills + DRAM writes (no roundtrip
        # stalls inside the Vector/GpSimd FIFOs)
        # the host ships rel_k_table pre-padded with replicated edge
        # columns, so W = qm @ T_ext.T directly yields the clip-padded row
        wr_insts = []
        for m in range(NT):
            ps = psum.tile([PT, S], F32, tag="ps", name=f"wps{m}")
            ps = ps[:, 0:DHALF]
            nc.tensor.matmul(ps[:], qmT_sb[:, m * PT:(m + 1) * PT], tT_sb[:],
                             start=True, stop=True)
            wp = wppool.tile([PT, WPADW], F16, tag="wpad", name=f"wpad{m}")
            nc.vector.tensor_copy(wp[:], ps[:, 0:WPADW])
            nc.vector.tensor_copy(w0_sb[m][:], ps[:, MAXREL - 1:MAXREL])
            nc.vector.tensor_copy(w256_sb[m][:],
                                  ps[:, MAXREL + NREL - 2:MAXREL + NREL - 1])
            wr_insts.append(nc.gpsimd.dma_start(wpad_d[m * PT:(m + 1) * PT, :],
                                                wp[:]))
        # Phase B: all band skew-reads, pipelined behind their writes
        # band[p, jj] = wpad[m*128+p, (jlo+jj)-(m*128+p)+255]
        bt_sb = []
        for m in range(NT):
            jlo, jhi = band_info[m]
            bt = bpool.tile([PT, jhi - jlo], F16, tag=f"band{m}",
                            name=f"band{m}")
            srcap = bass.AP(wpad_d, PT * (WPADW - 1) * m + jlo + (WPADW // 2),
                            [[WPADW - 1, PT], [1, jhi - jlo]])
            ri = nc.gpsimd.dma_start(bt[:], srcap)
            add_dep_helper(ri.ins, wr_insts[m].ins, reason="wpad DRAM RAW")
            bt_sb.append(bt)

        # Phase C: comb[m] = (mask-1)*MASKV + relbias (band + clipped edges);
        # emitted after proj0 so its band-read waits don't head-of-line-block
        # the projection evacuations in the Vector FIFO
        with tc.tile_wait_until(0.025):
            for m in range(NT):
                jlo, jhi = band_info[m]
                cb = comb_sb[m]
                nc.vector.tensor_add(cb[:, jlo:jhi], cb[:, jlo:jhi],
                                     bt_sb[m][:])
                if jlo > 0:
                    nc.vector.tensor_scalar_add(cb[:, 0:jlo], cb[:, 0:jlo],
                                                w0_sb[m][:])
                if jhi < S:
                    nc.vector.tensor_scalar_add(cb[:, jhi:S], cb[:, jhi:S],
                                                w256_sb[m][:])

        # ---- main loop: K=64 QK matmuls in alternating PE row groups
        # (ABAB order, distinct PSUM banks -> concurrent pairs); comb bias
        # accumulated via full K=128 identity matmuls (mixing row groups
        # inside one accumulation group crashes TRN2, so the injection
        # stays in 128x128 mode like the baseline) ----
        for t in range(4):
            for m in range(NT):
                mb = slice(m * PT, (m + 1) * PT)
                psA = psum.tile([PT, S], F32, tag="ps", name=f"psA{t}_{m}")
                psB = psum.tile([PT, S], F32, tag="ps", name=f"psB{t}_{m}")
                n0, n1 = slice(0, DHALF), slice(DHALF, S)
                cb = comb_sb[m]
                nc.tensor.matmul(psA[:, n0], qT_sb[t][:, mb], kTA_sb[t][:, n0],
                                 start=True, stop=False)
                nc.tensor.matmul(psB[:, n0], qT_sb[t][:, mb], kTB_sb[t][:, n0],
                                 start=True, stop=False)
                nc.tensor.matmul(psA[:, n1], qT_sb[t][:, mb], kTA_sb[t][:, n1],
                                 start=True, stop=False)
                nc.tensor.matmul(psB[:, n1], qT_sb[t][:, mb], kTB_sb[t][:, n1],
                                 start=True, stop=False)
                nc.tensor.matmul(psA[:, n0], id_sb[:], cb[:, n0],
                                 start=False, stop=True)
                nc.tensor.matmul(psB[:, n0], id_sb[:], cb[:, n0],
                                 start=False, stop=True)
                nc.tensor.matmul(psA[:, n1], id_sb[:], cb[:, n1],
                                 start=False, stop=True)
                nc.tensor.matmul(psB[:, n1], id_sb[:], cb[:, n1],
                                 start=False, stop=True)
                for hi_i, psx in ((0, psA), (1, psB)):
                    h = 2 * t + hi_i
                    e = epool.tile([PT, S], F16, tag="e", name=f"e{h}_{m}")
                    sm = spool.tile([PT, 1], F32, tag="s", name=f"s{h}_{m}")
                    nc.scalar.activation(e[:], psx[:], AF.Exp, bias=0.0,
                                         scale=1.0, accum_out=sm[:])
                    r = spool.tile([PT, 1], F32, tag="r", name=f"r{h}_{m}")
                    nc.vector.reciprocal(r[:], sm[:])
                    o = opool.tile([PT, S], F16, tag="o", name=f"o{h}_{m}")
                    nc.vector.tensor_scalar_mul(o[:], e[:], r[:])
                    nc.sync.dma_start(out_d[h, mb, :], o[:])
                # next pair's projections fill PE gaps (4 chunks per t-loop)
                if t < 3 and m in (1, 3, 5, 7):
                    ci = (m - 1) // 2
                    if ci < 2:
                        project_half(t + 1, ci, wq_tiles, x_tiles,
                                     qT_sb[t + 1], bq_sb, "q")
                    else:
                        project_half(t + 1, ci - 2, wk_tiles, k_tiles,
                                     (kTA_sb[t + 1], kTB_sb[t + 1]), bk_sb,
                                     "k")

    nc.compile()
    return nc


def _get_nc():
    if "nc" not in _CACHE:
        _CACHE["nc"] = _build()
    return _CACHE["nc"]


def _prep_inputs(query, key, mask, Wq, bq, Wk, bk, rel_k_table):
    """Host-side sharding prep -> 8 per-core input dicts."""
    sc = 1.0 / np.sqrt(np.float32(DK))
    query = np.asarray(query, dtype=np.float32)
    key = np.asarray(key, dtype=np.float32)
    maskf = np.ascontiguousarray(
        ((np.asarray(mask) != 0).astype(np.float32) - 1.0) * MASKV
    ).astype(np.float16)
    Wq = np.asarray(Wq, dtype=np.float32)
    bq = np.asarray(bq, dtype=np.float32)
    Wk = np.asarray(Wk, dtype=np.float32)
    bk = np.asarray(bk, dtype=np.float32)
    T = np.asarray(rel_k_table, dtype=np.float32)

    WqTs = np.ascontiguousarray((Wq * sc).T)       # [D, D]
    WkT = np.ascontiguousarray(Wk.T)               # [D, D]
    bqs = bq * sc
    Wm16 = np.ascontiguousarray(((Wq.reshape(H, DK, D).mean(0) * sc).T).astype(np.float16))
    bm = (bq.reshape(H, DK).mean(0) * sc).reshape(DK, 1).astype(np.float32)
    tTc16 = np.zeros((DK, DHALF), np.float16)  # [64, 512] edge-padded
    tTc16[:, 0:MAXREL - 1] = T.T[:, 0:1].astype(np.float16)
    tTc16[:, MAXREL - 1:MAXREL - 1 + NREL] = T.T.astype(np.float16)
    tTc16[:, MAXREL - 1 + NREL:WPADW] = T.T[:, NREL - 1:NREL].astype(np.float16)

    xT = [np.ascontiguousarray(query[b].T.astype(np.float16)) for b in range(B)]
    kT = [np.ascontiguousarray(key[b].T.astype(np.float16)) for b in range(B)]

    in_maps = []
    for c in range(NCORES):
        b, hh = divmod(c, 2)
        cols = slice(hh * DHALF, (hh + 1) * DHALF)
        in_maps.append(dict(
            xT=xT[b], kTx=kT[b], maskf=maskf[b],
            wqT=np.ascontiguousarray(WqTs[:, cols].astype(np.float16)),
            wkT=np.ascontiguousarray(WkT[:, cols].astype(np.float16)),
            bq4=np.ascontiguousarray(bqs[cols].reshape(4, PT).T),
            bk4=np.ascontiguousarray(bk[cols].reshape(4, PT).T),
            wmT=Wm16, bm1=bm, tT=tTc16,
        ))
    return in_maps


def run(inputs: dict, trace: bool = False):
    from concourse.bass_utils import run_bass_kernel_spmd

    nc = _get_nc()
    in_maps = _prep_inputs(**inputs)
    res = run_bass_kernel_spmd(nc, in_maps, core_ids=list(range(NCORES)),
                               trace=trace)
    out = np.empty((B, H, S, S), dtype=np.float32)
    for c in range(NCORES):
        b, hh = divmod(c, 2)
        out[b, hh * HPC:(hh + 1) * HPC] = res.results[c]["out"]
    return out, res


def kernel(**inputs) -> np.ndarray:
    out, _ = run(inputs)
    return out
